# revision 24
# baseline (speedup 1.0000x reference)
"""NeRD pixel decoder (SIREN MLP over 5x5 local patches) on 8 trn2 cores.

Sharding: row-shard the pixel dim. Core c handles image b=c//4, rows
y0=(c%4)*32 .. y0+32 (4096 pixels). The 5x5 patch extraction is folded into
the first matmul as 25 shifted-window matmuls against a zero-padded slab
(rows with 2-halo, cols padded by 2 on each side). SIREN weights replicated.

All matmul operands are bf16 (PSUM accumulation stays fp32): same PE rate as
fp32r at N=512 but half the DMA/SBUF traffic; fp8 DoubleRow measures the same
216ns/matmul as bf16 on HW, so bf16 is the right dtype at this accuracy.
Input DMAs are split into consumption-order chunks spread over the
SP/Activation/Pool DMA queues; ap=128 warmup matmuls ramp the PE DVFS p-state
while the first chunks land.

PSUM is organized as two [128, 2048] quad-bank tiles that ping-pong between
4-tile groups; each sin() is then ONE activation instruction over 2048
elements, keeping the activation engine off the critical path in the
L1/L2/head phase. The coords term is rank-1 per half (wc (x) gx + wc (x) gy):
two tiny K=1 outer-product matmuls compute it once, and the DVE pre-writes it
into each PSUM quad before the group's 25 shifted matmuls accumulate on top
(start=False), so no coords work sits between the last matmul and the sin.
"""

import numpy as np

FC = 128      # feature channels
P = 5         # patch
HID = 256
OUT = 3
OMEGA = 30.0
B, H, W = 2, 128, 128
NCORES = 8
ROWS = H // 4            # 32 image rows per core
NPIX = ROWS * W          # 4096 pixels per core
SLABR = ROWS + 4         # 36 slab rows (2 halo each side)
SLABW = W + 4            # 132 slab cols (2 pad each side)
TP = 512                 # pixels per PSUM bank tile (= 4 image rows)
NT = NPIX // TP          # 8 tiles per core
HB = 4                   # tiles per group (one quad-bank PSUM tile)
NWARM = 16               # warmup matmuls (p-state ramp while DMAs land)

_BUILT = {}


def _build():
    import concourse.tile as tile
    import concourse.mybir as mybir
    from concourse import bacc

    if "nc" in _BUILT:
        return _BUILT["nc"]

    f32 = mybir.dt.float32
    bf16 = mybir.dt.bfloat16
    Sin = mybir.ActivationFunctionType.Sin

    nc = bacc.Bacc("TRN2", target_bir_lowering=False, debug=False)

    xs = nc.dram_tensor("xs", [128, SLABR * SLABW], bf16, kind="ExternalInput").ap()
    w0a = nc.dram_tensor("w0a", [128, 25 * 128], bf16, kind="ExternalInput").ap()
    w0b = nc.dram_tensor("w0b", [128, 25 * 128], bf16, kind="ExternalInput").ap()
    # cog: [gx_row(128), gy(32), wcx_m0(128), wcx_m1(128), wcy_m0(128), wcy_m1(128)]
    cog = nc.dram_tensor("cog", [1, 672], bf16, kind="ExternalInput").ap()
    w123 = nc.dram_tensor("w123", [128, 8 * 128 + 2 * OUT], bf16,
                          kind="ExternalInput").ap()
    b012 = nc.dram_tensor("b012", [128, 6], f32, kind="ExternalInput").ap()
    b3 = nc.dram_tensor("b3", [OUT, 1], f32, kind="ExternalInput").ap()
    out = nc.dram_tensor("out", [OUT, NPIX], f32, kind="ExternalOutput").ap()

    RW = SLABW

    with tile.TileContext(nc) as tc:
        with (
            tc.tile_pool(name="const", bufs=1) as cpool,
            tc.tile_pool(name="h", bufs=2) as hpool,
            tc.tile_pool(name="osb", bufs=1) as opool,
            tc.tile_pool(name="ps", bufs=4, space="PSUM") as pspool,
        ):
            # ---- loads: consumption-order chunks over 3 DMA queues ----
            xs_t = cpool.tile([128, SLABR * SLABW], bf16, tag="xs", name="xs_t")
            w0a_t = cpool.tile([128, 25 * 128], bf16, tag="w0a", name="w0a_t")
            w0b_t = cpool.tile([128, 25 * 128], bf16, tag="w0b", name="w0b_t")
            cog_t = cpool.tile([1, 672], bf16, tag="cog", name="cog_t")
            w123_t = cpool.tile([128, 8 * 128 + 2 * OUT], bf16, tag="w123",
                                name="w123_t")
            b012_t = cpool.tile([128, 6], f32, tag="b012", name="b012_t")
            b3_t = cpool.tile([OUT, 1], f32, tag="b3", name="b3_t")
            wz_t = cpool.tile([128, 128], bf16, tag="wz", name="wz_t")
            zc_sb = cpool.tile([128, 2 * 160], f32, tag="zc", name="zc_sb")
            zc_full = cpool.tile([128, 2 * NPIX], f32, tag="zcf", name="zc_full")

            # SP queue: coords first, then alternating xs row chunks
            nc.sync.dma_start(cog_t[:], cog[:])
            nc.sync.dma_start(xs_t[:, 0:4 * RW], xs[:, 0:4 * RW])
            nc.sync.dma_start(xs_t[:, 8 * RW:12 * RW], xs[:, 8 * RW:12 * RW])
            nc.sync.dma_start(xs_t[:, 16 * RW:20 * RW], xs[:, 16 * RW:20 * RW])
            nc.sync.dma_start(b012_t[:], b012[:])
            # Activation queue: first w0 chunk tiny, interleaved with xs rows
            nc.scalar.dma_start(w0a_t[:, 0:1 * 128], w0a[:, 0:1 * 128])
            nc.scalar.dma_start(xs_t[:, 4 * RW:8 * RW], xs[:, 4 * RW:8 * RW])
            nc.scalar.dma_start(w0a_t[:, 1 * 128:4 * 128], w0a[:, 1 * 128:4 * 128])
            nc.scalar.dma_start(xs_t[:, 12 * RW:16 * RW], xs[:, 12 * RW:16 * RW])
            # Pool queue: rest of w0 (m=0), xs tail, w0 (m=1), later layers
            nc.gpsimd.memset(wz_t[:], 0.0)
            nc.gpsimd.dma_start(w0a_t[:, 4 * 128:9 * 128], w0a[:, 4 * 128:9 * 128])
            nc.gpsimd.dma_start(w0a_t[:, 9 * 128:25 * 128], w0a[:, 9 * 128:25 * 128])
            nc.gpsimd.dma_start(xs_t[:, 20 * RW:36 * RW], xs[:, 20 * RW:36 * RW])
            nc.gpsimd.dma_start(w0b_t[:], w0b[:])
            nc.gpsimd.dma_start(w123_t[:], w123[:])
            nc.gpsimd.dma_start(b3_t[:], b3[:])

            xs_r = xs_t[:].rearrange("p (r c) -> p r c", c=SLABW)

            def w0_chunk(m, o):
                t = w0a_t if m == 0 else w0b_t
                return t[:, o * 128:(o + 1) * 128]

            def rhs_l0(t, o):
                dy, dx = divmod(o, 5)
                return xs_r[:, 4 * t + dy: 4 * t + dy + 4, dx: dx + W]

            def h_slice(h, k, t, nt=1):
                return h[:, k * NPIX + t * TP: k * NPIX + (t + nt) * TP]

            # ---- warmup: ramp the PE p-state on zeros while DMAs stream ----
            warm_h = cpool.tile([128, 1], f32, tag="warm_h", name="warm_h")
            nc.scalar.activation(warm_h[:], wz_t[:, 0:1], Sin, bias=0.0, scale=1.0)
            for _ in range(NWARM):
                wps = pspool.tile([128, 128], f32, tag="ps", name="ps_warm")
                nc.tensor.matmul(wps[:], wz_t[:], wz_t[:], start=True, stop=True)

            # ---- coords: rank-1 outer products wc (x) gx|gy, K=1 matmuls ----
            for m in range(2):
                zps = pspool.tile([128, 2 * TP], f32, tag="ps", name=f"ps_zc_{m}")
                nc.tensor.matmul(zps[:, 0:128],
                                 cog_t[0:1, 160 + m * 128:160 + (m + 1) * 128],
                                 cog_t[0:1, 0:128], start=True, stop=True)
                nc.tensor.matmul(zps[:, TP:TP + 32],
                                 cog_t[0:1, 416 + m * 128:416 + (m + 1) * 128],
                                 cog_t[0:1, 128:160], start=True, stop=True)
                nc.vector.tensor_copy(zc_sb[:, m * 160:m * 160 + 128],
                                      zps[:, 0:128])
                nc.vector.tensor_copy(zc_sb[:, m * 160 + 128:(m + 1) * 160],
                                      zps[:, TP:TP + 32])

            # materialize the full coords plane once per half on the DVE:
            # zc_full[m] = wcx_m (x) gx  +  wcy_m (x) gy   over all 32 rows
            for m in range(2):
                zf = zc_full[:, m * NPIX:(m + 1) * NPIX] \
                    .rearrange("p (r x) -> p r x", x=W)
                A_b = zc_sb[:, m * 160:m * 160 + 128] \
                    .rearrange("p (r x) -> p r x", r=1).broadcast_to([128, ROWS, W])
                g_b = zc_sb[:, m * 160 + 128:m * 160 + 160] \
                    .rearrange("p (r x) -> p r x", x=1).broadcast_to([128, ROWS, W])
                nc.vector.tensor_add(zf, A_b, g_b)

            # ---- layer 0: 25 shifted matmuls + coords add, sin ----
            h0 = hpool.tile([128, 2 * NPIX], bf16, tag="h", name="h0")
            for m in range(2):
                for th in (0, HB):
                    psd = [pspool.tile([128, 2 * TP], f32, tag="ps",
                                       name=f"ps_l0_{m}_{th}_{p}")
                           for p in range(2)]
                    for o in range(25):
                        for ti in range(HB):
                            nc.tensor.matmul(
                                psd[ti // 2][:, (ti % 2) * TP:(ti % 2 + 1) * TP],
                                w0_chunk(m, o), rhs_l0(th + ti, o),
                                start=(o == 0), stop=(o == 24))
                    for p in range(2):
                        t0 = th + 2 * p
                        nc.vector.tensor_add(
                            psd[p][:], psd[p][:],
                            zc_full[:, m * NPIX + t0 * TP:m * NPIX + (t0 + 2) * TP])
                        nc.scalar.activation(
                            h_slice(h0, m, t0, 2), psd[p][:], Sin,
                            bias=b012_t[:, m:m + 1], scale=OMEGA)

            # ---- layers 1, 2: dense 256->256, sin ----
            hin = h0
            for li in range(2):
                bl_t = b012_t[:, 2 + 2 * li: 4 + 2 * li]
                wl_t = w123_t[:, li * 4 * 128:(li + 1) * 4 * 128]
                hout = hpool.tile([128, 2 * NPIX], bf16, tag="h", name=f"h{li+1}")
                for m in range(2):
                    for th in (0, HB):
                        psd = [pspool.tile([128, 2 * TP], f32, tag="ps",
                                           name=f"ps_l{li+1}_{m}_{th}_{p}")
                               for p in range(2)]
                        for k in range(2):
                            for ti in range(HB):
                                nc.tensor.matmul(
                                    psd[ti // 2][:, (ti % 2) * TP:(ti % 2 + 1) * TP],
                                    wl_t[:, (k * 2 + m) * 128:(k * 2 + m + 1) * 128],
                                    h_slice(hin, k, th + ti),
                                    start=(k == 0), stop=(k == 1))
                        for p in range(2):
                            nc.scalar.activation(
                                h_slice(hout, m, th + 2 * p, 2), psd[p][:], Sin,
                                bias=bl_t[:, m:m + 1], scale=OMEGA)
                hin = hout

            # ---- head: 256 -> 3, + bias, streamed out per quad ----
            w3_t = w123_t[:, 8 * 128:8 * 128 + 2 * OUT]
            out_sb = opool.tile([OUT, NPIX], f32, tag="osb", name="out_sb")
            Identity = mybir.ActivationFunctionType.Identity
            for th in (0, HB):
                psd = [pspool.tile([OUT, 2 * TP], f32, tag="ps",
                                   name=f"ps_hd_{th}_{p}")
                       for p in range(2)]
                for k in range(2):
                    for ti in range(HB):
                        nc.tensor.matmul(
                            psd[ti // 2][:, (ti % 2) * TP:(ti % 2 + 1) * TP],
                            w3_t[:, k * OUT:(k + 1) * OUT],
                            h_slice(hin, k, th + ti), start=(k == 0), stop=(k == 1))
                for p in range(2):
                    t0 = th + 2 * p
                    if p == 0:
                        nc.vector.tensor_scalar_add(
                            out_sb[:, t0 * TP:(t0 + 2) * TP], psd[p][:],
                            b3_t[:, 0:1])
                    else:
                        nc.scalar.activation(
                            out_sb[:, t0 * TP:(t0 + 2) * TP], psd[p][:],
                            Identity, bias=b3_t[:, 0:1], scale=1.0)
                    nc.sync.dma_start(out[:, t0 * TP:(t0 + 2) * TP],
                                      out_sb[:, t0 * TP:(t0 + 2) * TP])

    nc.finalize()
    _BUILT["nc"] = nc
    return nc


def _prep_core_inputs(c, xi_bf, gx_row, ys_all, wc_h):
    import ml_dtypes
    bf = ml_dtypes.bfloat16
    b = c // 4
    y0 = (c % 4) * ROWS
    slab = np.zeros((128, SLABR, SLABW), bf)
    ylo, yhi = y0 - 2, y0 + ROWS + 2
    slo, shi = max(ylo, 0), min(yhi, H)
    slab[:, slo - ylo: shi - ylo, 2:2 + W] = xi_bf[b, :, slo:shi, :]

    gy = ys_all[y0:y0 + ROWS]
    cog = np.concatenate([gx_row, gy, wc_h[0], wc_h[1]]).astype(bf).reshape(1, 672)
    return {"xs": slab.reshape(128, SLABR * SLABW), "cog": cog}


def kernel(**inputs):
    import ml_dtypes
    from concourse.bass_utils import run_bass_kernel_spmd

    bf = ml_dtypes.bfloat16
    xi = np.asarray(inputs["xi"], np.float32)
    W0 = np.asarray(inputs["W0"], np.float32)
    b0 = np.asarray(inputs["b0"], np.float32)
    W1 = np.asarray(inputs["W1"], np.float32)
    b1 = np.asarray(inputs["b1"], np.float32)
    W2 = np.asarray(inputs["W2"], np.float32)
    b2 = np.asarray(inputs["b2"], np.float32)
    W3 = np.asarray(inputs["W3"], np.float32)
    b3 = np.asarray(inputs["b3"], np.float32)

    # replicated weight tensors, rearranged for the PE (lhsT chunks)
    w0p = W0[:FC * P * P].reshape(128, 25, HID)          # [c, o, j]
    w0a_h = np.ascontiguousarray(w0p[:, :, :128]).reshape(128, 25 * 128).astype(bf)
    w0b_h = np.ascontiguousarray(w0p[:, :, 128:]).reshape(128, 25 * 128).astype(bf)
    wc_h = W0[FC * P * P:]                               # [2, 256]
    w1_h = W1.reshape(2, 128, 2, 128).transpose(1, 0, 2, 3).reshape(128, 512)
    w2_h = W2.reshape(2, 128, 2, 128).transpose(1, 0, 2, 3).reshape(128, 512)
    w3_h = W3.reshape(2, 128, OUT).transpose(1, 0, 2).reshape(128, 2 * OUT)
    w123_h = np.concatenate([w1_h, w2_h, w3_h], axis=1).astype(bf)
    b012_h = np.ascontiguousarray(np.concatenate(
        [(OMEGA * b).reshape(2, 128).T for b in (b0, b1, b2)], axis=1))
    b3_h = np.ascontiguousarray(b3.reshape(OUT, 1))

    # normalized coords, matching jnp.linspace/meshgrid in the reference
    ys_all = np.linspace(-1.0, 1.0, H, dtype=np.float32)
    gx_row = np.linspace(-1.0, 1.0, W, dtype=np.float32)

    xi_bf = xi.astype(bf)

    shared = {
        "w0a": w0a_h, "w0b": w0b_h, "w123": w123_h,
        "b012": b012_h, "b3": b3_h,
    }
    in_maps = []
    for c in range(NCORES):
        m = _prep_core_inputs(c, xi_bf, gx_row, ys_all, wc_h)
        m.update(shared)
        in_maps.append(m)

    nc = _build()
    res = run_bass_kernel_spmd(nc, in_maps, core_ids=list(range(NCORES)))

    full = np.empty((B, OUT, H, W), np.float32)
    for c in range(NCORES):
        b = c // 4
        y0 = (c % 4) * ROWS
        full[b, :, y0:y0 + ROWS, :] = res.results[c]["out"].reshape(OUT, ROWS, W)
    return full


# revision 26
# speedup vs baseline: 1.1773x; 1.1773x over previous
"""NeRD pixel decoder (SIREN MLP over 5x5 local patches) on 8 trn2 cores.

Sharding: row-shard the pixel dim. Core c handles image b=c//4, rows
y0=(c%4)*32 .. y0+32 (4096 pixels). The 5x5 patch extraction is folded into
the first matmul as 25 shifted-window matmuls against a zero-padded slab
(rows with 2-halo, cols padded by 2 on each side). SIREN weights replicated.

All matmul operands are bf16 (PSUM accumulation stays fp32): same PE rate as
fp32r at N=512 but half the DMA/SBUF traffic; fp8 DoubleRow measures the same
216ns/matmul as bf16 on HW, so bf16 is the right dtype at this accuracy.
Input DMAs are split into consumption-order chunks spread over the
SP/Activation/Pool DMA queues; ap=128 warmup matmuls ramp the PE DVFS p-state
while the first chunks land.

PSUM is organized as two [128, 2048] quad-bank tiles that ping-pong between
4-tile groups; each sin() is then ONE activation instruction over 2048
elements, keeping the activation engine off the critical path in the
L1/L2/head phase. The coords term is rank-1 per half (wc (x) gx + wc (x) gy):
two tiny K=1 outer-product matmuls compute it once, and the DVE pre-writes it
into each PSUM quad before the group's 25 shifted matmuls accumulate on top
(start=False), so no coords work sits between the last matmul and the sin.
"""

import numpy as np

FC = 128      # feature channels
P = 5         # patch
HID = 256
OUT = 3
OMEGA = 30.0
B, H, W = 2, 128, 128
NCORES = 8
ROWS = H // 4            # 32 image rows per core
NPIX = ROWS * W          # 4096 pixels per core
SLABR = ROWS + 4         # 36 slab rows (2 halo each side)
SLABW = W + 4            # 132 slab cols (2 pad each side)
TP = 512                 # pixels per PSUM bank tile (= 4 image rows)
NT = NPIX // TP          # 8 tiles per core
HB = 4                   # tiles per group (one quad-bank PSUM tile)
NWARM = 22               # warmup matmuls (p-state ramp while DMAs land)

_BUILT = {}


def _build():
    import concourse.tile as tile
    import concourse.mybir as mybir
    from concourse import bacc

    if "nc" in _BUILT:
        return _BUILT["nc"]

    f32 = mybir.dt.float32
    bf16 = mybir.dt.bfloat16
    Sin = mybir.ActivationFunctionType.Sin

    nc = bacc.Bacc("TRN2", target_bir_lowering=False, debug=False)

    xs = nc.dram_tensor("xs", [128, SLABR * SLABW], bf16, kind="ExternalInput").ap()
    w0a = nc.dram_tensor("w0a", [128, 25 * 128], bf16, kind="ExternalInput").ap()
    w0b = nc.dram_tensor("w0b", [128, 25 * 128], bf16, kind="ExternalInput").ap()
    # cog: [gx_row(128), gy(32), wcx_m0(128), wcx_m1(128), wcy_m0(128), wcy_m1(128)]
    cog = nc.dram_tensor("cog", [1, 672], bf16, kind="ExternalInput").ap()
    w123 = nc.dram_tensor("w123", [128, 8 * 128 + 2 * OUT], bf16,
                          kind="ExternalInput").ap()
    b012 = nc.dram_tensor("b012", [128, 6], f32, kind="ExternalInput").ap()
    b3 = nc.dram_tensor("b3", [OUT, 1], f32, kind="ExternalInput").ap()
    out = nc.dram_tensor("out", [OUT, NPIX], f32, kind="ExternalOutput").ap()

    RW = SLABW

    with tile.TileContext(nc) as tc:
        with (
            tc.tile_pool(name="const", bufs=1) as cpool,
            tc.tile_pool(name="h", bufs=2) as hpool,
            tc.tile_pool(name="osb", bufs=1) as opool,
            tc.tile_pool(name="ps", bufs=4, space="PSUM") as pspool,
        ):
            # ---- loads: consumption-order chunks over 3 DMA queues ----
            xs_t = cpool.tile([128, SLABR * SLABW], bf16, tag="xs", name="xs_t")
            w0a_t = cpool.tile([128, 25 * 128], bf16, tag="w0a", name="w0a_t")
            w0b_t = cpool.tile([128, 25 * 128], bf16, tag="w0b", name="w0b_t")
            cog_t = cpool.tile([1, 672], bf16, tag="cog", name="cog_t")
            w123_t = cpool.tile([128, 8 * 128 + 2 * OUT], bf16, tag="w123",
                                name="w123_t")
            b012_t = cpool.tile([128, 6], f32, tag="b012", name="b012_t")
            b3_t = cpool.tile([OUT, 1], f32, tag="b3", name="b3_t")
            wz_t = cpool.tile([128, 128], bf16, tag="wz", name="wz_t")
            zc_sb = cpool.tile([128, 2 * 160], f32, tag="zc", name="zc_sb")
            zc_full = cpool.tile([128, 2 * NPIX], f32, tag="zcf", name="zc_full")

            # SP queue: coords first, then the first xs rows
            nc.sync.dma_start(cog_t[:], cog[:])
            nc.sync.dma_start(xs_t[:, 0:4 * RW], xs[:, 0:4 * RW])
            nc.sync.dma_start(xs_t[:, 4 * RW:8 * RW], xs[:, 4 * RW:8 * RW])
            nc.sync.dma_start(xs_t[:, 16 * RW:20 * RW], xs[:, 16 * RW:20 * RW])
            nc.sync.dma_start(b012_t[:], b012[:])
            # Activation queue: first w0 chunks + middle xs rows
            nc.scalar.dma_start(w0a_t[:, 0:3 * 128], w0a[:, 0:3 * 128])
            nc.scalar.dma_start(xs_t[:, 8 * RW:12 * RW], xs[:, 8 * RW:12 * RW])
            nc.scalar.dma_start(xs_t[:, 12 * RW:16 * RW], xs[:, 12 * RW:16 * RW])
            # Pool queue: rest of w0 (m=0), xs tail, w0 (m=1), later layers
            nc.gpsimd.memset(wz_t[:], 0.0)
            nc.gpsimd.dma_start(w0a_t[:, 3 * 128:8 * 128], w0a[:, 3 * 128:8 * 128])
            nc.gpsimd.dma_start(w0a_t[:, 8 * 128:25 * 128], w0a[:, 8 * 128:25 * 128])
            nc.gpsimd.dma_start(xs_t[:, 20 * RW:36 * RW], xs[:, 20 * RW:36 * RW])
            nc.gpsimd.dma_start(w0b_t[:], w0b[:])
            nc.gpsimd.dma_start(w123_t[:], w123[:])
            nc.gpsimd.dma_start(b3_t[:], b3[:])

            xs_r = xs_t[:].rearrange("p (r c) -> p r c", c=SLABW)

            def w0_chunk(m, o):
                t = w0a_t if m == 0 else w0b_t
                return t[:, o * 128:(o + 1) * 128]

            def rhs_l0(t, o):
                dy, dx = divmod(o, 5)
                return xs_r[:, 4 * t + dy: 4 * t + dy + 4, dx: dx + W]

            def h_slice(h, k, t, nt=1):
                return h[:, k * NPIX + t * TP: k * NPIX + (t + nt) * TP]

            # ---- warmup: ramp the PE p-state on zeros while DMAs stream ----
            warm_h = cpool.tile([128, 1], f32, tag="warm_h", name="warm_h")
            nc.scalar.activation(warm_h[:], wz_t[:, 0:1], Sin, bias=0.0, scale=1.0)
            for _ in range(NWARM):
                wps = pspool.tile([128, 128], f32, tag="ps", name="ps_warm")
                nc.tensor.matmul(wps[:], wz_t[:], wz_t[:], start=True, stop=True)

            # ---- coords: rank-1 outer products wc (x) gx|gy, K=1 matmuls ----
            for m in range(2):
                zps = pspool.tile([128, 2 * TP], f32, tag="ps", name=f"ps_zc_{m}")
                nc.tensor.matmul(zps[:, 0:128],
                                 cog_t[0:1, 160 + m * 128:160 + (m + 1) * 128],
                                 cog_t[0:1, 0:128], start=True, stop=True)
                nc.tensor.matmul(zps[:, TP:TP + 32],
                                 cog_t[0:1, 416 + m * 128:416 + (m + 1) * 128],
                                 cog_t[0:1, 128:160], start=True, stop=True)
                nc.vector.tensor_copy(zc_sb[:, m * 160:m * 160 + 128],
                                      zps[:, 0:128])
                nc.vector.tensor_copy(zc_sb[:, m * 160 + 128:(m + 1) * 160],
                                      zps[:, TP:TP + 32])

            # materialize the full coords plane once per half on the DVE:
            # zc_full[m] = wcx_m (x) gx  +  wcy_m (x) gy   over all 32 rows
            for m in range(2):
                zf = zc_full[:, m * NPIX:(m + 1) * NPIX] \
                    .rearrange("p (r x) -> p r x", x=W)
                A_b = zc_sb[:, m * 160:m * 160 + 128] \
                    .rearrange("p (r x) -> p r x", r=1).broadcast_to([128, ROWS, W])
                g_b = zc_sb[:, m * 160 + 128:m * 160 + 160] \
                    .rearrange("p (r x) -> p r x", x=1).broadcast_to([128, ROWS, W])
                nc.vector.tensor_add(zf, A_b, g_b)

            # ---- layer 0: 25 shifted matmuls + coords add, sin ----
            h0 = hpool.tile([128, 2 * NPIX], bf16, tag="h", name="h0")
            for m in range(2):
                for th in (0, HB):
                    psd = [pspool.tile([128, 2 * TP], f32, tag="ps",
                                       name=f"ps_l0_{m}_{th}_{p}")
                           for p in range(2)]
                    for o in range(25):
                        for ti in range(HB):
                            nc.tensor.matmul(
                                psd[ti // 2][:, (ti % 2) * TP:(ti % 2 + 1) * TP],
                                w0_chunk(m, o), rhs_l0(th + ti, o),
                                start=(o == 0), stop=(o == 24))
                    for p in range(2):
                        t0 = th + 2 * p
                        nc.vector.tensor_add(
                            psd[p][:], psd[p][:],
                            zc_full[:, m * NPIX + t0 * TP:m * NPIX + (t0 + 2) * TP])
                        nc.scalar.activation(
                            h_slice(h0, m, t0, 2), psd[p][:], Sin,
                            bias=b012_t[:, m:m + 1], scale=OMEGA)

            # ---- layers 1, 2: dense 256->256, sin ----
            hin = h0
            for li in range(2):
                bl_t = b012_t[:, 2 + 2 * li: 4 + 2 * li]
                wl_t = w123_t[:, li * 4 * 128:(li + 1) * 4 * 128]
                hout = hpool.tile([128, 2 * NPIX], bf16, tag="h", name=f"h{li+1}")
                for m in range(2):
                    for th in (0, HB):
                        psd = [pspool.tile([128, 2 * TP], f32, tag="ps",
                                           name=f"ps_l{li+1}_{m}_{th}_{p}")
                               for p in range(2)]
                        for k in range(2):
                            for ti in range(HB):
                                nc.tensor.matmul(
                                    psd[ti // 2][:, (ti % 2) * TP:(ti % 2 + 1) * TP],
                                    wl_t[:, (k * 2 + m) * 128:(k * 2 + m + 1) * 128],
                                    h_slice(hin, k, th + ti),
                                    start=(k == 0), stop=(k == 1))
                        for p in range(2):
                            nc.scalar.activation(
                                h_slice(hout, m, th + 2 * p, 2), psd[p][:], Sin,
                                bias=bl_t[:, m:m + 1], scale=OMEGA)
                hin = hout

            # ---- head: 256 -> 3, + bias, streamed out per quad ----
            w3_t = w123_t[:, 8 * 128:8 * 128 + 2 * OUT]
            out_sb = opool.tile([OUT, NPIX], f32, tag="osb", name="out_sb")
            Identity = mybir.ActivationFunctionType.Identity
            for th in (0, HB):
                psd = [pspool.tile([OUT, 2 * TP], f32, tag="ps",
                                   name=f"ps_hd_{th}_{p}")
                       for p in range(2)]
                for k in range(2):
                    for ti in range(HB):
                        nc.tensor.matmul(
                            psd[ti // 2][:, (ti % 2) * TP:(ti % 2 + 1) * TP],
                            w3_t[:, k * OUT:(k + 1) * OUT],
                            h_slice(hin, k, th + ti), start=(k == 0), stop=(k == 1))
                for p in range(2):
                    t0 = th + 2 * p
                    if p == 0:
                        nc.vector.tensor_scalar_add(
                            out_sb[:, t0 * TP:(t0 + 2) * TP], psd[p][:],
                            b3_t[:, 0:1])
                    else:
                        nc.scalar.activation(
                            out_sb[:, t0 * TP:(t0 + 2) * TP], psd[p][:],
                            Identity, bias=b3_t[:, 0:1], scale=1.0)
                    nc.sync.dma_start(out[:, t0 * TP:(t0 + 2) * TP],
                                      out_sb[:, t0 * TP:(t0 + 2) * TP])

    nc.finalize()
    _BUILT["nc"] = nc
    return nc


def _prep_core_inputs(c, xi_bf, gx_row, ys_all, wc_h):
    import ml_dtypes
    bf = ml_dtypes.bfloat16
    b = c // 4
    y0 = (c % 4) * ROWS
    slab = np.zeros((128, SLABR, SLABW), bf)
    ylo, yhi = y0 - 2, y0 + ROWS + 2
    slo, shi = max(ylo, 0), min(yhi, H)
    slab[:, slo - ylo: shi - ylo, 2:2 + W] = xi_bf[b, :, slo:shi, :]

    gy = ys_all[y0:y0 + ROWS]
    cog = np.concatenate([gx_row, gy, wc_h[0], wc_h[1]]).astype(bf).reshape(1, 672)
    return {"xs": slab.reshape(128, SLABR * SLABW), "cog": cog}


def kernel(**inputs):
    import ml_dtypes
    from concourse.bass_utils import run_bass_kernel_spmd

    bf = ml_dtypes.bfloat16
    xi = np.asarray(inputs["xi"], np.float32)
    W0 = np.asarray(inputs["W0"], np.float32)
    b0 = np.asarray(inputs["b0"], np.float32)
    W1 = np.asarray(inputs["W1"], np.float32)
    b1 = np.asarray(inputs["b1"], np.float32)
    W2 = np.asarray(inputs["W2"], np.float32)
    b2 = np.asarray(inputs["b2"], np.float32)
    W3 = np.asarray(inputs["W3"], np.float32)
    b3 = np.asarray(inputs["b3"], np.float32)

    # replicated weight tensors, rearranged for the PE (lhsT chunks)
    w0p = W0[:FC * P * P].reshape(128, 25, HID)          # [c, o, j]
    w0a_h = np.ascontiguousarray(w0p[:, :, :128]).reshape(128, 25 * 128).astype(bf)
    w0b_h = np.ascontiguousarray(w0p[:, :, 128:]).reshape(128, 25 * 128).astype(bf)
    wc_h = W0[FC * P * P:]                               # [2, 256]
    w1_h = W1.reshape(2, 128, 2, 128).transpose(1, 0, 2, 3).reshape(128, 512)
    w2_h = W2.reshape(2, 128, 2, 128).transpose(1, 0, 2, 3).reshape(128, 512)
    w3_h = W3.reshape(2, 128, OUT).transpose(1, 0, 2).reshape(128, 2 * OUT)
    w123_h = np.concatenate([w1_h, w2_h, w3_h], axis=1).astype(bf)
    b012_h = np.ascontiguousarray(np.concatenate(
        [(OMEGA * b).reshape(2, 128).T for b in (b0, b1, b2)], axis=1))
    b3_h = np.ascontiguousarray(b3.reshape(OUT, 1))

    # normalized coords, matching jnp.linspace/meshgrid in the reference
    ys_all = np.linspace(-1.0, 1.0, H, dtype=np.float32)
    gx_row = np.linspace(-1.0, 1.0, W, dtype=np.float32)

    xi_bf = xi.astype(bf)

    shared = {
        "w0a": w0a_h, "w0b": w0b_h, "w123": w123_h,
        "b012": b012_h, "b3": b3_h,
    }
    in_maps = []
    for c in range(NCORES):
        m = _prep_core_inputs(c, xi_bf, gx_row, ys_all, wc_h)
        m.update(shared)
        in_maps.append(m)

    nc = _build()
    res = run_bass_kernel_spmd(nc, in_maps, core_ids=list(range(NCORES)))

    full = np.empty((B, OUT, H, W), np.float32)
    for c in range(NCORES):
        b = c // 4
        y0 = (c % 4) * ROWS
        full[b, :, y0:y0 + ROWS, :] = res.results[c]["out"].reshape(OUT, ROWS, W)
    return full
